# revision 4
# baseline (speedup 1.0000x reference)
"""Trainium2 Bass kernel for the CachedParamMgr scatter_memory problem.

Reference semantics (see problem): given a CPU weight table [1M,128], a GPU row
cache [200k,128], and 65536 lookup ids (all cache misses by construction), the
hot path evicts the 65536 highest-cpu-id slots, writes the evicted rows back to
the weight table, fetches the missing rows from the weight table into the freed
slots, updates the idx maps, and returns the gathered rows.

Strategy (8 NeuronCores, SPMD, no collectives — host routes, device moves):
  * Host computes the tiny index plan (top-k over 200k int32s, map updates)
    with exact reference semantics.
  * For the structural fast path (warm identity cache + ids a permutation of
    [200000,265536), as the reference setup guarantees) every row movement is
    either a contiguous block copy or a gather from a 64k-row window:
      - weight_out = weight_in, except [134464,200000) <- cache_in rows (same
        indices: block copy)
      - cache_out  = cache_in, except slots [134464,200000) <- fetched rows in
        descending-j order
      - out        = weight_in[cpu_row_idxs]  (row gather, j order)
    Work is split uniformly: each core copies 1/8 of the untouched weight and
    cache rows, 1/8 of the evicted-row writeback, and gathers its 8192-row
    j-block with two windowed dma_gathers (int16 idx limit) + bitwise select.
  * All output blocks come back as contiguous per-core pieces; the host only
    concatenates slices (the descending-slot block is a view flip).
  * Falls back to a pure-host computation if the structure doesn't hold.
"""
import sys

sys.path.insert(0, "/opt/trn_rl_repo")

import numpy as np

NUM_EMB = 1_000_000
DIM = 128
CUDA_ROWS = 200_000
K = 65_536
E0 = CUDA_ROWS - K          # 134464: eviction slot range [E0, CUDA_ROWS)
F0 = CUDA_ROWS              # 200000: fast-path id range [F0, F1)
F1 = CUDA_ROWS + K          # 265536
NCORE = 8
JPC = K // NCORE            # 8192 ids per core
S = JPC // 128              # 64 free slots in the gather dst tile
UWA = E0 // NCORE           # 16808 untouched weight rows/core from [0, E0)
UWB = (NUM_EMB - F0) // NCORE   # 100000 untouched weight rows/core from [F0, 1M)
WHALF = 32_768              # gather window half (int16 index limit)

_COMPILED = {}


def _host_plan(ids, idx_map, cached_idx_map, inverted_cached_idx):
    """Exact-reference index computation (numpy). Returns the plan dict."""
    cpu_row_idxs = idx_map[ids]
    backlist = np.zeros(NUM_EMB, dtype=bool)
    backlist[cpu_row_idxs] = True
    masked = np.where(backlist[cached_idx_map], np.int32(-2), cached_idx_map)
    # jax.lax.top_k: values descending, ties broken by lowest index
    evict = np.argsort(-masked, kind="stable")[: cpu_row_idxs.shape[0]].astype(np.int32)
    evict_info = cached_idx_map[evict]
    cached_new = cached_idx_map.copy()
    inv_new = inverted_cached_idx.copy()
    cached_new[evict] = -1
    inv_new[evict_info] = -1
    cached_new[evict] = cpu_row_idxs
    inv_new[cpu_row_idxs] = evict
    gpu_row_idxs = inv_new[cpu_row_idxs]
    return dict(
        cpu_row_idxs=cpu_row_idxs,
        evict=evict,
        evict_info=evict_info,
        cached_new=cached_new,
        inv_new=inv_new,
        gpu_row_idxs=gpu_row_idxs,
    )


def _fast_path_ok(plan, cached_idx_map):
    cri = plan["cpu_row_idxs"]
    if cri.shape[0] != K:
        return False
    if not np.array_equal(np.sort(cri), np.arange(F0, F1, dtype=cri.dtype)):
        return False
    if not np.array_equal(cached_idx_map, np.arange(CUDA_ROWS, dtype=cached_idx_map.dtype)):
        return False
    return True


def _host_fallback(weight, cache, plan):
    """Pure-host reference computation of the row data (general path)."""
    evict, evict_info, cri = plan["evict"], plan["evict_info"], plan["cpu_row_idxs"]
    w = weight.copy()
    cc = cache.copy()
    w[evict_info] = cache[evict]
    fetch = w[cri]
    cc[evict] = fetch
    out = cc[plan["gpu_row_idxs"]]
    return out, cc, w


def _wrap_idx(idx):
    """int16 idx list [n] -> [128, n//16] wrapped layout (i%16, i//16), x8."""
    arr = idx.reshape(-1, 16).T.astype(np.int16)
    return np.ascontiguousarray(np.tile(arr, (8, 1)))


def _build_program():
    import concourse.tile as tile
    from concourse import bacc, mybir

    nc = bacc.Bacc("TRN2", target_bir_lowering=False, debug=False)
    f32, i32, i16 = mybir.dt.float32, mybir.dt.int32, mybir.dt.int16

    def din(name, shape, dt=f32):
        return nc.dram_tensor(name, shape, dt, kind="ExternalInput").ap()

    def dout(name, shape, dt=f32):
        return nc.dram_tensor(name, shape, dt, kind="ExternalOutput").ap()

    wa = din("wa", [UWA, DIM])          # untouched weight slice from [0, E0)
    wb = din("wb", [UWB, DIM])          # untouched weight slice from [F0, 1M)
    ca = din("ca", [UWA, DIM])          # untouched cache slice from [0, E0)
    cs = din("cs", [JPC, DIM])          # cache slot block (evicted rows)
    ga = din("ga", [WHALF, DIM])        # gather window A = weight[F0:F0+32768]
    gb = din("gb", [WHALF, DIM])        # gather window B = weight[F0+32768:F1]
    idxa = din("idxa", [128, JPC // 16], i16)
    idxb = din("idxb", [128, JPC // 16], i16)
    msk = din("msk", [128, S * DIM], i32)

    wa_o = dout("wa_o", [UWA, DIM])
    wb_o = dout("wb_o", [UWB, DIM])
    ca_o = dout("ca_o", [UWA, DIM])
    we_o = dout("we_o", [JPC, DIM])     # weight rows [slot block] <- cs
    cs_o = dout("cs_o", [JPC, DIM])     # fetched rows, j order (host flips)
    out_o = dout("out_o", [JPC, DIM])   # fetched rows, j order

    # Bulk copies must go through SWDGE (gpsimd): concurrent HWDGE bulk DMA
    # traffic alongside dma_gather wedges the device (verified empirically).
    hw = [nc.gpsimd, nc.gpsimd]
    with tile.TileContext(nc) as tc:
        with tc.tile_pool(name="p", bufs=1) as pool:
            ia = pool.tile([128, JPC // 16], i16, tag="ia")
            ib = pool.tile([128, JPC // 16], i16, tag="ib")
            m = pool.tile([128, S * DIM], i32, tag="m")
            da = pool.tile([128, S, DIM], f32, tag="da")
            db = pool.tile([128, S, DIM], f32, tag="db")
            t = pool.tile([128, S * DIM], i32, tag="t")

            nc.sync.dma_start(ia[:], idxa[:])
            nc.sync.dma_start(ib[:], idxb[:])
            nc.sync.dma_start(m[:], msk[:])
            # dma_gather hangs/fails above ~1024 idxs per instruction (HW
            # descriptor-ring limit; verified empirically) — chunk at 1024
            GC = 1024
            cc, cslot = GC // 16, GC // 128
            for k in range(JPC // GC):
                nc.gpsimd.dma_gather(
                    da[:, k * cslot:(k + 1) * cslot, :], ga[:],
                    ia[:, k * cc:(k + 1) * cc], GC, GC, DIM)
                nc.gpsimd.dma_gather(
                    db[:, k * cslot:(k + 1) * cslot, :], gb[:],
                    ib[:, k * cc:(k + 1) * cc], GC, GC, DIM)
            dai = da[:].rearrange("p s d -> p (s d)").bitcast(i32)
            dbi = db[:].rearrange("p s d -> p (s d)").bitcast(i32)
            x = mybir.AluOpType.bitwise_xor
            nc.vector.tensor_tensor(t[:], dai, dbi, op=x)
            nc.vector.tensor_tensor(t[:], t[:], m[:], op=mybir.AluOpType.bitwise_and)
            nc.vector.tensor_tensor(dai, t[:], dai, op=x)
            nc.sync.dma_start(out_o.rearrange("(s p) d -> p s d", p=128), da[:])
            nc.scalar.dma_start(cs_o.rearrange("(s p) d -> p s d", p=128), da[:])

            # bulk contiguous copies, chunked across the two HWDGE issuers
            def copy_chunks(dst, src, rows, nchunk, eoff=0):
                step = rows // nchunk
                assert step * nchunk == rows
                for i in range(nchunk):
                    sl = slice(i * step, (i + 1) * step)
                    hw[(eoff + i) % 2].dma_start(dst[sl], src[sl])

            copy_chunks(we_o, cs, JPC, 1)
            copy_chunks(wa_o, wa, UWA, 2)
            copy_chunks(ca_o, ca, UWA, 2, eoff=1)
            copy_chunks(wb_o, wb, UWB, 8)
    nc.compile()
    return nc


def _get_program():
    if "nc" not in _COMPILED:
        _COMPILED["nc"] = _build_program()
    return _COMPILED["nc"]


def kernel(weight, cuda_cached_weight, ids, idx_map, cached_idx_map, inverted_cached_idx):
    from concourse.bass_utils import run_bass_kernel_spmd

    weight = np.asarray(weight, dtype=np.float32)
    cache = np.asarray(cuda_cached_weight, dtype=np.float32)
    ids = np.asarray(ids, dtype=np.int32)
    idx_map = np.asarray(idx_map, dtype=np.int32)
    cached_idx_map = np.asarray(cached_idx_map, dtype=np.int32)
    inverted_cached_idx = np.asarray(inverted_cached_idx, dtype=np.int32)

    plan = _host_plan(ids, idx_map, cached_idx_map, inverted_cached_idx)

    if not _fast_path_ok(plan, cached_idx_map):
        out, cc, w = _host_fallback(weight, cache, plan)
        return out, cc, w, plan["cached_new"], plan["inv_new"]

    cri = plan["cpu_row_idxs"]
    nc = _get_program()

    ga_v = weight[F0 : F0 + WHALF]
    gb_v = weight[F0 + WHALF : F1]
    in_maps = []
    for c in range(NCORE):
        src = cri[c * JPC : (c + 1) * JPC] - F0   # window-relative, [0, 65536)
        in_b = src >= WHALF
        idxa_v = _wrap_idx(np.where(in_b, 0, src).astype(np.int16))
        idxb_v = _wrap_idx((np.where(in_b, src - WHALF, 0)).astype(np.int16))
        flags = np.where(in_b, np.int32(-1), np.int32(0)).reshape(S, 128).T
        msk_v = np.ascontiguousarray(np.repeat(flags, DIM, axis=1))
        sb = E0 + (NCORE - 1 - c) * JPC           # this core's slot block start
        in_maps.append({
            "wa": weight[c * UWA : (c + 1) * UWA],
            "wb": weight[F0 + c * UWB : F0 + (c + 1) * UWB],
            "ca": cache[c * UWA : (c + 1) * UWA],
            "cs": cache[sb : sb + JPC],
            "ga": ga_v,
            "gb": gb_v,
            "idxa": idxa_v,
            "idxb": idxb_v,
            "msk": msk_v,
        })

    res = run_bass_kernel_spmd(nc, in_maps, list(range(NCORE)), trace=False)
    r = res.results

    weight_out = np.empty_like(weight)
    cache_out = np.empty_like(cache)
    out = np.empty((K, DIM), dtype=np.float32)
    for c in range(NCORE):
        sb = E0 + (NCORE - 1 - c) * JPC
        weight_out[c * UWA : (c + 1) * UWA] = r[c]["wa_o"]
        weight_out[F0 + c * UWB : F0 + (c + 1) * UWB] = r[c]["wb_o"]
        weight_out[sb : sb + JPC] = r[c]["we_o"]
        cache_out[c * UWA : (c + 1) * UWA] = r[c]["ca_o"]
        cache_out[sb : sb + JPC] = r[c]["cs_o"][::-1]   # descending-j slots
        out[c * JPC : (c + 1) * JPC] = r[c]["out_o"]

    return out, cache_out, weight_out, plan["cached_new"], plan["inv_new"]


# revision 7
# speedup vs baseline: 1.1221x; 1.1221x over previous
"""Trainium2 Bass kernel for the CachedParamMgr scatter_memory problem.

Reference semantics (see problem): given a CPU weight table [1M,128], a GPU row
cache [200k,128], and 65536 lookup ids (all cache misses by construction), the
hot path evicts the 65536 highest-cpu-id slots, writes the evicted rows back to
the weight table, fetches the missing rows from the weight table into the freed
slots, updates the idx maps, and returns the gathered rows.

Strategy (8 NeuronCores, SPMD, no collectives — host routes, device moves):
  * Host computes the tiny index plan (top-k over 200k int32s, map updates)
    with exact reference semantics.
  * For the structural fast path (warm identity cache + ids a permutation of
    [200000,265536), as the reference setup guarantees) every row movement is
    either a contiguous block copy or a gather from a 64k-row window:
      - weight_out = weight_in, except [134464,200000) <- cache_in rows (same
        indices: block copy)
      - cache_out  = cache_in, except slots [134464,200000) <- fetched rows in
        descending-j order
      - out        = weight_in[cpu_row_idxs]  (row gather, j order)
    Work is split uniformly: each core copies 1/8 of the untouched weight and
    cache rows, 1/8 of the evicted-row writeback, and gathers its 8192-row
    j-block with two windowed dma_gathers (int16 idx limit) + bitwise select.
  * All output blocks come back as contiguous per-core pieces; the host only
    concatenates slices (the descending-slot block is a view flip).
  * Falls back to a pure-host computation if the structure doesn't hold.
"""
import sys

sys.path.insert(0, "/opt/trn_rl_repo")

import numpy as np

NUM_EMB = 1_000_000
DIM = 128
CUDA_ROWS = 200_000
K = 65_536
E0 = CUDA_ROWS - K          # 134464: eviction slot range [E0, CUDA_ROWS)
F0 = CUDA_ROWS              # 200000: fast-path id range [F0, F1)
F1 = CUDA_ROWS + K          # 265536
NCORE = 8
JPC = K // NCORE            # 8192 ids per core
S = JPC // 128              # 64 free slots in the gather dst tile
UWA = E0 // NCORE           # 16808 untouched weight rows/core from [0, E0)
UWB = (NUM_EMB - F0) // NCORE   # 100000 untouched weight rows/core from [F0, 1M)
WHALF = 32_768              # gather window half (int16 index limit)

_COMPILED = {}


def _host_plan(ids, idx_map, cached_idx_map, inverted_cached_idx):
    """Exact-reference index computation (numpy). Returns the plan dict."""
    cpu_row_idxs = idx_map[ids]
    backlist = np.zeros(NUM_EMB, dtype=bool)
    backlist[cpu_row_idxs] = True
    masked = np.where(backlist[cached_idx_map], np.int32(-2), cached_idx_map)
    # jax.lax.top_k: values descending, ties broken by lowest index
    evict = np.argsort(-masked, kind="stable")[: cpu_row_idxs.shape[0]].astype(np.int32)
    evict_info = cached_idx_map[evict]
    cached_new = cached_idx_map.copy()
    inv_new = inverted_cached_idx.copy()
    cached_new[evict] = -1
    inv_new[evict_info] = -1
    cached_new[evict] = cpu_row_idxs
    inv_new[cpu_row_idxs] = evict
    gpu_row_idxs = inv_new[cpu_row_idxs]
    return dict(
        cpu_row_idxs=cpu_row_idxs,
        evict=evict,
        evict_info=evict_info,
        cached_new=cached_new,
        inv_new=inv_new,
        gpu_row_idxs=gpu_row_idxs,
    )


def _fast_path_ok(plan, cached_idx_map):
    cri = plan["cpu_row_idxs"]
    if cri.shape[0] != K:
        return False
    if not np.array_equal(np.sort(cri), np.arange(F0, F1, dtype=cri.dtype)):
        return False
    if not np.array_equal(cached_idx_map, np.arange(CUDA_ROWS, dtype=cached_idx_map.dtype)):
        return False
    return True


def _host_fallback(weight, cache, plan):
    """Pure-host reference computation of the row data (general path)."""
    evict, evict_info, cri = plan["evict"], plan["evict_info"], plan["cpu_row_idxs"]
    w = weight.copy()
    cc = cache.copy()
    w[evict_info] = cache[evict]
    fetch = w[cri]
    cc[evict] = fetch
    out = cc[plan["gpu_row_idxs"]]
    return out, cc, w


def _wrap_idx(idx):
    """int16 idx list [n] -> [128, n//16] wrapped layout (i%16, i//16), x8."""
    arr = idx.reshape(-1, 16).T.astype(np.int16)
    return np.ascontiguousarray(np.tile(arr, (8, 1)))


def _build_program():
    import concourse.tile as tile
    from concourse import bacc, mybir

    nc = bacc.Bacc("TRN2", target_bir_lowering=False, debug=False)
    f32, i32, i16 = mybir.dt.float32, mybir.dt.int32, mybir.dt.int16

    def din(name, shape, dt=f32):
        return nc.dram_tensor(name, shape, dt, kind="ExternalInput").ap()

    def dout(name, shape, dt=f32):
        return nc.dram_tensor(name, shape, dt, kind="ExternalOutput").ap()

    wa = din("wa", [UWA, DIM])          # untouched weight slice from [0, E0)
    wb = din("wb", [UWB, DIM])          # untouched weight slice from [F0, 1M)
    ca = din("ca", [UWA, DIM])          # untouched cache slice from [0, E0)
    cs = din("cs", [JPC, DIM])          # cache slot block (evicted rows)
    # gather window weight[F0:F1] viewed as 32768 superrows of 2 rows — keeps
    # indices within the int16 dma_gather limit with a single index list
    ga = din("ga", [WHALF, 2 * DIM])
    idxa = din("idxa", [128, JPC // 16], i16)
    msk = din("msk", [128, S * DIM], i32)

    wa_o = dout("wa_o", [UWA, DIM])
    wb_o = dout("wb_o", [UWB, DIM])
    ca_o = dout("ca_o", [UWA, DIM])
    we_o = dout("we_o", [JPC, DIM])     # weight rows [slot block] <- cs
    cs_o = dout("cs_o", [JPC, DIM])     # fetched rows, j order (host flips)
    out_o = dout("out_o", [JPC, DIM])   # fetched rows, j order

    # Bulk copies must go through SWDGE (gpsimd): concurrent HWDGE bulk DMA
    # traffic alongside dma_gather wedges the device (verified empirically).
    hw = [nc.gpsimd, nc.gpsimd]
    with tile.TileContext(nc) as tc:
        with tc.tile_pool(name="p", bufs=1) as pool:
            ia = pool.tile([128, JPC // 16], i16, tag="ia")
            m = pool.tile([128, S * DIM], i32, tag="m")
            dg = pool.tile([128, S, 2 * DIM], f32, tag="dg")
            dm = pool.tile([128, S, DIM], f32, tag="dm")
            t = pool.tile([128, S * DIM], i32, tag="t")

            # bulk contiguous copies FIRST: their data movement overlaps the
            # (serialized, ~8.6us/1024-idx) Q7 gather descriptor generation
            def copy_chunks(dst, src, rows, nchunk, eoff=0):
                step = rows // nchunk
                assert step * nchunk == rows
                for i in range(nchunk):
                    sl = slice(i * step, (i + 1) * step)
                    hw[(eoff + i) % 2].dma_start(dst[sl], src[sl])

            copy_chunks(we_o, cs, JPC, 1)
            copy_chunks(wa_o, wa, UWA, 2)
            copy_chunks(ca_o, ca, UWA, 2, eoff=1)
            copy_chunks(wb_o, wb, UWB, 8)

            nc.sync.dma_start(ia[:], idxa[:])
            nc.sync.dma_start(m[:], msk[:])
            # dma_gather hangs/fails above ~1024 idxs per instruction (HW
            # descriptor-ring limit; verified empirically) — chunk at 1024
            GC = 1024
            cc, cslot = GC // 16, GC // 128
            for k in range(JPC // GC):
                nc.gpsimd.dma_gather(
                    dg[:, k * cslot:(k + 1) * cslot, :], ga[:],
                    ia[:, k * cc:(k + 1) * cc], GC, GC, 2 * DIM)
            # select even/odd row half of each gathered superrow (bitwise,
            # exact): dm = A ^ ((A ^ B) & m)
            a3 = dg[:, :, 0:DIM].bitcast(i32)
            b3 = dg[:, :, DIM:2 * DIM].bitcast(i32)
            t3 = t[:].rearrange("p (s d) -> p s d", d=DIM)
            m3 = m[:].rearrange("p (s d) -> p s d", d=DIM)
            dm3 = dm[:].bitcast(i32)
            x = mybir.AluOpType.bitwise_xor
            nc.vector.tensor_tensor(t3, a3, b3, op=x)
            nc.vector.tensor_tensor(t3, t3, m3, op=mybir.AluOpType.bitwise_and)
            nc.vector.tensor_tensor(dm3, t3, a3, op=x)
            nc.sync.dma_start(out_o.rearrange("(s p) d -> p s d", p=128), dm[:])
            nc.scalar.dma_start(cs_o.rearrange("(s p) d -> p s d", p=128), dm[:])
    nc.compile()
    return nc


def _get_program():
    if "nc" not in _COMPILED:
        _COMPILED["nc"] = _build_program()
    return _COMPILED["nc"]


def kernel(weight, cuda_cached_weight, ids, idx_map, cached_idx_map, inverted_cached_idx):
    from concourse.bass_utils import run_bass_kernel_spmd

    weight = np.asarray(weight, dtype=np.float32)
    cache = np.asarray(cuda_cached_weight, dtype=np.float32)
    ids = np.asarray(ids, dtype=np.int32)
    idx_map = np.asarray(idx_map, dtype=np.int32)
    cached_idx_map = np.asarray(cached_idx_map, dtype=np.int32)
    inverted_cached_idx = np.asarray(inverted_cached_idx, dtype=np.int32)

    plan = _host_plan(ids, idx_map, cached_idx_map, inverted_cached_idx)

    if not _fast_path_ok(plan, cached_idx_map):
        out, cc, w = _host_fallback(weight, cache, plan)
        return out, cc, w, plan["cached_new"], plan["inv_new"]

    cri = plan["cpu_row_idxs"]
    nc = _get_program()

    ga_v = weight[F0:F1].reshape(WHALF, 2 * DIM)   # superrow view (free)
    in_maps = []
    for c in range(NCORE):
        src = cri[c * JPC : (c + 1) * JPC] - F0   # window-relative, [0, 65536)
        idxa_v = _wrap_idx((src >> 1).astype(np.int16))
        odd = (src & 1).astype(bool)
        flags = np.where(odd, np.int32(-1), np.int32(0)).reshape(S, 128).T
        msk_v = np.ascontiguousarray(np.repeat(flags, DIM, axis=1))
        sb = E0 + (NCORE - 1 - c) * JPC           # this core's slot block start
        in_maps.append({
            "wa": weight[c * UWA : (c + 1) * UWA],
            "wb": weight[F0 + c * UWB : F0 + (c + 1) * UWB],
            "ca": cache[c * UWA : (c + 1) * UWA],
            "cs": cache[sb : sb + JPC],
            "ga": ga_v,
            "idxa": idxa_v,
            "msk": msk_v,
        })

    res = run_bass_kernel_spmd(nc, in_maps, list(range(NCORE)), trace=False)
    r = res.results

    weight_out = np.empty_like(weight)
    cache_out = np.empty_like(cache)
    out = np.empty((K, DIM), dtype=np.float32)
    for c in range(NCORE):
        sb = E0 + (NCORE - 1 - c) * JPC
        weight_out[c * UWA : (c + 1) * UWA] = r[c]["wa_o"]
        weight_out[F0 + c * UWB : F0 + (c + 1) * UWB] = r[c]["wb_o"]
        weight_out[sb : sb + JPC] = r[c]["we_o"]
        cache_out[c * UWA : (c + 1) * UWA] = r[c]["ca_o"]
        cache_out[sb : sb + JPC] = r[c]["cs_o"][::-1]   # descending-j slots
        out[c * JPC : (c + 1) * JPC] = r[c]["out_o"]

    return out, cache_out, weight_out, plan["cached_new"], plan["inv_new"]


# revision 10
# speedup vs baseline: 1.4736x; 1.3132x over previous
"""Trainium2 Bass kernel for the CachedParamMgr scatter_memory problem.

Reference semantics (see problem): given a CPU weight table [1M,128], a GPU row
cache [200k,128], and 65536 lookup ids (all cache misses by construction), the
hot path evicts the 65536 highest-cpu-id slots, writes the evicted rows back to
the weight table, fetches the missing rows from the weight table into the freed
slots, updates the idx maps, and returns the gathered rows.

Strategy (8 NeuronCores, SPMD, no collectives — host routes, device moves):
  * Host computes the tiny index plan (top-k over 200k int32s, map updates)
    with exact reference semantics.
  * For the structural fast path (warm identity cache + ids a permutation of
    [200000,265536), as the reference setup guarantees) every row movement is
    either a contiguous block copy or a gather from a 64k-row window:
      - weight_out = weight_in, except [134464,200000) <- cache_in rows (same
        indices: block copy)
      - cache_out  = cache_in, except slots [134464,200000) <- fetched rows in
        descending-j order
      - out        = weight_in[cpu_row_idxs]  (row gather, j order)
    Work is split uniformly: each core copies 1/8 of the untouched weight and
    cache rows, 1/8 of the evicted-row writeback, and gathers its 8192-row
    j-block with two windowed dma_gathers (int16 idx limit) + bitwise select.
  * All output blocks come back as contiguous per-core pieces; the host only
    concatenates slices (the descending-slot block is a view flip).
  * Falls back to a pure-host computation if the structure doesn't hold.
"""
import sys

sys.path.insert(0, "/opt/trn_rl_repo")

import numpy as np

NUM_EMB = 1_000_000
DIM = 128
CUDA_ROWS = 200_000
K = 65_536
E0 = CUDA_ROWS - K          # 134464: eviction slot range [E0, CUDA_ROWS)
F0 = CUDA_ROWS              # 200000: fast-path id range [F0, F1)
F1 = CUDA_ROWS + K          # 265536
NCORE = 8
JPC = K // NCORE            # 8192 ids per core
S = JPC // 128              # 64 free slots in the gather dst tile
UWA = E0 // NCORE           # 16808 untouched weight rows/core from [0, E0)
UWB = (NUM_EMB - F0) // NCORE   # 100000 untouched weight rows/core from [F0, 1M)
WHALF = 32_768              # gather window half (int16 index limit)

_COMPILED = {}


def _host_plan(ids, idx_map, cached_idx_map, inverted_cached_idx):
    """Exact-reference index computation (numpy). Returns the plan dict."""
    cpu_row_idxs = idx_map[ids]
    backlist = np.zeros(NUM_EMB, dtype=bool)
    backlist[cpu_row_idxs] = True
    masked = np.where(backlist[cached_idx_map], np.int32(-2), cached_idx_map)
    # jax.lax.top_k: values descending, ties broken by lowest index
    evict = np.argsort(-masked, kind="stable")[: cpu_row_idxs.shape[0]].astype(np.int32)
    evict_info = cached_idx_map[evict]
    cached_new = cached_idx_map.copy()
    inv_new = inverted_cached_idx.copy()
    cached_new[evict] = -1
    inv_new[evict_info] = -1
    cached_new[evict] = cpu_row_idxs
    inv_new[cpu_row_idxs] = evict
    gpu_row_idxs = inv_new[cpu_row_idxs]
    return dict(
        cpu_row_idxs=cpu_row_idxs,
        evict=evict,
        evict_info=evict_info,
        cached_new=cached_new,
        inv_new=inv_new,
        gpu_row_idxs=gpu_row_idxs,
    )


def _fast_path_ok(plan, cached_idx_map):
    cri = plan["cpu_row_idxs"]
    if cri.shape[0] != K:
        return False
    if not np.array_equal(np.sort(cri), np.arange(F0, F1, dtype=cri.dtype)):
        return False
    if not np.array_equal(cached_idx_map, np.arange(CUDA_ROWS, dtype=cached_idx_map.dtype)):
        return False
    return True


def _host_fallback(weight, cache, plan):
    """Pure-host reference computation of the row data (general path)."""
    evict, evict_info, cri = plan["evict"], plan["evict_info"], plan["cpu_row_idxs"]
    w = weight.copy()
    cc = cache.copy()
    w[evict_info] = cache[evict]
    fetch = w[cri]
    cc[evict] = fetch
    out = cc[plan["gpu_row_idxs"]]
    return out, cc, w


def _wrap_idx(idx):
    """int16 idx list [n] -> [128, n//16] wrapped layout (i%16, i//16), x8."""
    arr = idx.reshape(-1, 16).T.astype(np.int16)
    return np.ascontiguousarray(np.tile(arr, (8, 1)))


def _build_program():
    import concourse.tile as tile
    from concourse import bacc, mybir

    nc = bacc.Bacc("TRN2", target_bir_lowering=False, debug=False)
    f32, i32, i16 = mybir.dt.float32, mybir.dt.int32, mybir.dt.int16

    def din(name, shape, dt=f32):
        return nc.dram_tensor(name, shape, dt, kind="ExternalInput").ap()

    def dout(name, shape, dt=f32):
        return nc.dram_tensor(name, shape, dt, kind="ExternalOutput").ap()

    wa = din("wa", [UWA, DIM])          # untouched weight slice from [0, E0)
    wb = din("wb", [UWB, DIM])          # untouched weight slice from [F0, 1M)
    ca = din("ca", [UWA, DIM])          # untouched cache slice from [0, E0)
    cs = din("cs", [JPC, DIM])          # cache slot block (evicted rows)
    # gather window weight[F0:F1] viewed as 32768 superrows of 2 rows — keeps
    # indices within the int16 dma_gather limit with a single index list
    ga = din("ga", [WHALF, 2 * DIM])
    idxa = din("idxa", [128, JPC // 16], i16)
    msk = din("msk", [128, S * DIM], i32)

    wa_o = dout("wa_o", [UWA, DIM])
    wb_o = dout("wb_o", [UWB, DIM])
    ca_o = dout("ca_o", [UWA, DIM])
    we_o = dout("we_o", [JPC, DIM])     # weight rows [slot block] <- cs
    out_o = dout("out_o", [JPC, DIM])   # fetched rows, j order

    # Bulk copies must go through SWDGE (gpsimd): concurrent HWDGE bulk DMA
    # traffic alongside dma_gather wedges the device (verified empirically).
    hw = [nc.gpsimd, nc.gpsimd]
    # dma_gather hangs/fails above ~1024 idxs per instruction (HW
    # descriptor-ring limit; verified empirically) — chunk at 1024
    GC = 1024
    NG = JPC // GC                      # 8 gather chunks
    cc, cslot = GC // 16, GC // 128
    with tile.TileContext(nc) as tc:
        with tc.tile_pool(name="p", bufs=1) as pool:
            ia = pool.tile([128, JPC // 16], i16, tag="ia")
            m = pool.tile([128, S * DIM], i32, tag="m")
            nc.sync.dma_start(ia[:], idxa[:])
            nc.sync.dma_start(m[:], msk[:])

            # bulk copy chunks, interleaved with the gathers on the gpsimd
            # queue: copy data keeps the SDMA engines fed while the Q7 grinds
            # through gather descriptor generation (~8.6us per 1024 idxs),
            # and the FIFO descriptor ring never backs up behind one giant
            # copy train.
            copies = []
            def add_chunks(dst, src, rows, nchunk):
                step = rows // nchunk
                assert step * nchunk == rows
                for i in range(nchunk):
                    sl = slice(i * step, (i + 1) * step)
                    copies.append((dst[sl], src[sl]))

            add_chunks(we_o, cs, JPC, 1)
            add_chunks(wa_o, wa, UWA, 2)
            add_chunks(ca_o, ca, UWA, 2)
            add_chunks(wb_o, wb, UWB, 8)

            x = mybir.AluOpType.bitwise_xor
            ci = 0
            for k in range(NG):
                if ci < len(copies):
                    nc.gpsimd.dma_start(*copies[ci]); ci += 1
                dg = pool.tile([128, cslot, 2 * DIM], f32, tag=f"dg{k}")
                dm = pool.tile([128, cslot, DIM], f32, tag=f"dm{k}")
                t = pool.tile([128, cslot, DIM], i32, tag=f"t{k}")
                nc.gpsimd.dma_gather(
                    dg[:], ga[:], ia[:, k * cc:(k + 1) * cc], GC, GC, 2 * DIM)
                # select even/odd row half of each gathered superrow
                # (bitwise, exact): dm = A ^ ((A ^ B) & m)
                a3 = dg[:, :, 0:DIM].bitcast(i32)
                b3 = dg[:, :, DIM:2 * DIM].bitcast(i32)
                m3 = m[:, k * cslot * DIM:(k + 1) * cslot * DIM].rearrange(
                    "p (s d) -> p s d", d=DIM)
                nc.vector.tensor_tensor(t[:], a3, b3, op=x)
                nc.vector.tensor_tensor(t[:], t[:], m3, op=mybir.AluOpType.bitwise_and)
                nc.vector.tensor_tensor(dm[:].bitcast(i32), t[:], a3, op=x)
                oc = out_o[k * GC:(k + 1) * GC]
                nc.sync.dma_start(oc.rearrange("(s p) d -> p s d", p=128), dm[:])
            while ci < len(copies):
                nc.gpsimd.dma_start(*copies[ci]); ci += 1
    nc.compile()
    return nc


def _get_program():
    if "nc" not in _COMPILED:
        _COMPILED["nc"] = _build_program()
    return _COMPILED["nc"]


def kernel(weight, cuda_cached_weight, ids, idx_map, cached_idx_map, inverted_cached_idx):
    from concourse.bass_utils import run_bass_kernel_spmd

    weight = np.asarray(weight, dtype=np.float32)
    cache = np.asarray(cuda_cached_weight, dtype=np.float32)
    ids = np.asarray(ids, dtype=np.int32)
    idx_map = np.asarray(idx_map, dtype=np.int32)
    cached_idx_map = np.asarray(cached_idx_map, dtype=np.int32)
    inverted_cached_idx = np.asarray(inverted_cached_idx, dtype=np.int32)

    plan = _host_plan(ids, idx_map, cached_idx_map, inverted_cached_idx)

    if not _fast_path_ok(plan, cached_idx_map):
        out, cc, w = _host_fallback(weight, cache, plan)
        return out, cc, w, plan["cached_new"], plan["inv_new"]

    cri = plan["cpu_row_idxs"]
    nc = _get_program()

    ga_v = weight[F0:F1].reshape(WHALF, 2 * DIM)   # superrow view (free)
    in_maps = []
    for c in range(NCORE):
        src = cri[c * JPC : (c + 1) * JPC] - F0   # window-relative, [0, 65536)
        idxa_v = _wrap_idx((src >> 1).astype(np.int16))
        odd = (src & 1).astype(bool)
        flags = np.where(odd, np.int32(-1), np.int32(0)).reshape(S, 128).T
        msk_v = np.ascontiguousarray(np.repeat(flags, DIM, axis=1))
        sb = E0 + (NCORE - 1 - c) * JPC           # this core's slot block start
        in_maps.append({
            "wa": weight[c * UWA : (c + 1) * UWA],
            "wb": weight[F0 + c * UWB : F0 + (c + 1) * UWB],
            "ca": cache[c * UWA : (c + 1) * UWA],
            "cs": cache[sb : sb + JPC],
            "ga": ga_v,
            "idxa": idxa_v,
            "msk": msk_v,
        })

    res = run_bass_kernel_spmd(nc, in_maps, list(range(NCORE)), trace=False)
    r = res.results

    weight_out = np.empty_like(weight)
    cache_out = np.empty_like(cache)
    out = np.empty((K, DIM), dtype=np.float32)
    for c in range(NCORE):
        sb = E0 + (NCORE - 1 - c) * JPC
        weight_out[c * UWA : (c + 1) * UWA] = r[c]["wa_o"]
        weight_out[F0 + c * UWB : F0 + (c + 1) * UWB] = r[c]["wb_o"]
        weight_out[sb : sb + JPC] = r[c]["we_o"]
        cache_out[c * UWA : (c + 1) * UWA] = r[c]["ca_o"]
        cache_out[sb : sb + JPC] = r[c]["out_o"][::-1]  # descending-j slots
        out[c * JPC : (c + 1) * JPC] = r[c]["out_o"]

    return out, cache_out, weight_out, plan["cached_new"], plan["inv_new"]


# revision 11
# speedup vs baseline: 1.4884x; 1.0101x over previous
"""Trainium2 Bass kernel for the CachedParamMgr scatter_memory problem.

Reference semantics (see problem): given a CPU weight table [1M,128], a GPU row
cache [200k,128], and 65536 lookup ids (all cache misses by construction), the
hot path evicts the 65536 highest-cpu-id slots, writes the evicted rows back to
the weight table, fetches the missing rows from the weight table into the freed
slots, updates the idx maps, and returns the gathered rows.

Strategy (8 NeuronCores, SPMD, no collectives — host routes, device moves):
  * Host computes the tiny index plan (top-k over 200k int32s, map updates)
    with exact reference semantics.
  * For the structural fast path (warm identity cache + ids a permutation of
    [200000,265536), as the reference setup guarantees) every row movement is
    either a contiguous block copy or a gather from a 64k-row window:
      - weight_out = weight_in, except [134464,200000) <- cache_in rows (same
        indices: block copy)
      - cache_out  = cache_in, except slots [134464,200000) <- fetched rows in
        descending-j order
      - out        = weight_in[cpu_row_idxs]  (row gather, j order)
    Work is split uniformly: each core copies 1/8 of the untouched weight and
    cache rows, 1/8 of the evicted-row writeback, and gathers its 8192-row
    j-block with two windowed dma_gathers (int16 idx limit) + bitwise select.
  * All output blocks come back as contiguous per-core pieces; the host only
    concatenates slices (the descending-slot block is a view flip).
  * Falls back to a pure-host computation if the structure doesn't hold.
"""
import sys

sys.path.insert(0, "/opt/trn_rl_repo")

import numpy as np

NUM_EMB = 1_000_000
DIM = 128
CUDA_ROWS = 200_000
K = 65_536
E0 = CUDA_ROWS - K          # 134464: eviction slot range [E0, CUDA_ROWS)
F0 = CUDA_ROWS              # 200000: fast-path id range [F0, F1)
F1 = CUDA_ROWS + K          # 265536
NCORE = 8
JPC = K // NCORE            # 8192 ids per core
S = JPC // 128              # 64 free slots in the gather dst tile
UWA = E0 // NCORE           # 16808 untouched weight rows/core from [0, E0)
UWB = (NUM_EMB - F0) // NCORE   # 100000 untouched weight rows/core from [F0, 1M)
WHALF = 32_768              # gather window half (int16 index limit)

_COMPILED = {}


def _host_plan(ids, idx_map, cached_idx_map, inverted_cached_idx):
    """Exact-reference index computation (numpy). Returns the plan dict."""
    cpu_row_idxs = idx_map[ids]
    backlist = np.zeros(NUM_EMB, dtype=bool)
    backlist[cpu_row_idxs] = True
    masked = np.where(backlist[cached_idx_map], np.int32(-2), cached_idx_map)
    # jax.lax.top_k: values descending, ties broken by lowest index
    evict = np.argsort(-masked, kind="stable")[: cpu_row_idxs.shape[0]].astype(np.int32)
    evict_info = cached_idx_map[evict]
    cached_new = cached_idx_map.copy()
    inv_new = inverted_cached_idx.copy()
    cached_new[evict] = -1
    inv_new[evict_info] = -1
    cached_new[evict] = cpu_row_idxs
    inv_new[cpu_row_idxs] = evict
    gpu_row_idxs = inv_new[cpu_row_idxs]
    return dict(
        cpu_row_idxs=cpu_row_idxs,
        evict=evict,
        evict_info=evict_info,
        cached_new=cached_new,
        inv_new=inv_new,
        gpu_row_idxs=gpu_row_idxs,
    )


def _fast_path_ok(plan, cached_idx_map):
    cri = plan["cpu_row_idxs"]
    if cri.shape[0] != K:
        return False
    if not np.array_equal(np.sort(cri), np.arange(F0, F1, dtype=cri.dtype)):
        return False
    if not np.array_equal(cached_idx_map, np.arange(CUDA_ROWS, dtype=cached_idx_map.dtype)):
        return False
    return True


def _host_fallback(weight, cache, plan):
    """Pure-host reference computation of the row data (general path)."""
    evict, evict_info, cri = plan["evict"], plan["evict_info"], plan["cpu_row_idxs"]
    w = weight.copy()
    cc = cache.copy()
    w[evict_info] = cache[evict]
    fetch = w[cri]
    cc[evict] = fetch
    out = cc[plan["gpu_row_idxs"]]
    return out, cc, w


def _wrap_idx(idx):
    """int16 idx list [n] -> [128, n//16] wrapped layout (i%16, i//16), x8."""
    arr = idx.reshape(-1, 16).T.astype(np.int16)
    return np.ascontiguousarray(np.tile(arr, (8, 1)))


def _build_program():
    import concourse.tile as tile
    from concourse import bacc, mybir

    nc = bacc.Bacc("TRN2", target_bir_lowering=False, debug=False)
    f32, i32, i16 = mybir.dt.float32, mybir.dt.int32, mybir.dt.int16

    def din(name, shape, dt=f32):
        return nc.dram_tensor(name, shape, dt, kind="ExternalInput").ap()

    def dout(name, shape, dt=f32):
        return nc.dram_tensor(name, shape, dt, kind="ExternalOutput").ap()

    wa = din("wa", [UWA, DIM])          # untouched weight slice from [0, E0)
    wb = din("wb", [UWB, DIM])          # untouched weight slice from [F0, 1M)
    ca = din("ca", [UWA, DIM])          # untouched cache slice from [0, E0)
    cs = din("cs", [JPC, DIM])          # cache slot block (evicted rows)
    # gather window weight[F0:F1] viewed as 32768 superrows of 2 rows — keeps
    # indices within the int16 dma_gather limit with a single index list
    ga = din("ga", [WHALF, 2 * DIM])
    idxa = din("idxa", [128, JPC // 16], i16)
    msk = din("msk", [128, S], i32)     # one select flag per gathered row

    wa_o = dout("wa_o", [UWA, DIM])
    wb_o = dout("wb_o", [UWB, DIM])
    ca_o = dout("ca_o", [UWA, DIM])
    we_o = dout("we_o", [JPC, DIM])     # weight rows [slot block] <- cs
    out_o = dout("out_o", [JPC, DIM])   # fetched rows, j order

    # Bulk copies must go through SWDGE (gpsimd): concurrent HWDGE bulk DMA
    # traffic alongside dma_gather wedges the device (verified empirically).
    hw = [nc.gpsimd, nc.gpsimd]
    # dma_gather hangs/fails above ~1024 idxs per instruction (HW
    # descriptor-ring limit; verified empirically) — chunk at 1024
    GC = 1024
    NG = JPC // GC                      # 8 gather chunks
    cc, cslot = GC // 16, GC // 128
    with tile.TileContext(nc) as tc:
        with tc.tile_pool(name="p", bufs=1) as pool:
            ia = pool.tile([128, JPC // 16], i16, tag="ia")
            m = pool.tile([128, S], i32, tag="m")
            nc.sync.dma_start(ia[:], idxa[:])
            nc.sync.dma_start(m[:], msk[:])

            # bulk copy chunks, interleaved with the gathers on the gpsimd
            # queue: copy data keeps the SDMA engines fed while the Q7 grinds
            # through gather descriptor generation (~8.6us per 1024 idxs),
            # and the FIFO descriptor ring never backs up behind one giant
            # copy train.
            copies = []
            def add_chunks(dst, src, rows, nchunk):
                step = rows // nchunk
                assert step * nchunk == rows
                for i in range(nchunk):
                    sl = slice(i * step, (i + 1) * step)
                    copies.append((dst[sl], src[sl]))

            add_chunks(we_o, cs, JPC, 1)
            add_chunks(wa_o, wa, UWA, 2)
            add_chunks(ca_o, ca, UWA, 2)
            add_chunks(wb_o, wb, UWB, 8)

            x = mybir.AluOpType.bitwise_xor
            ci = 0
            for k in range(NG):
                if ci < len(copies):
                    nc.gpsimd.dma_start(*copies[ci]); ci += 1
                dg = pool.tile([128, cslot, 2 * DIM], f32, tag=f"dg{k}")
                dm = pool.tile([128, cslot, DIM], f32, tag=f"dm{k}")
                t = pool.tile([128, cslot, DIM], i32, tag=f"t{k}")
                nc.gpsimd.dma_gather(
                    dg[:], ga[:], ia[:, k * cc:(k + 1) * cc], GC, GC, 2 * DIM)
                # select even/odd row half of each gathered superrow
                # (bitwise, exact): dm = A ^ ((A ^ B) & m)
                a3 = dg[:, :, 0:DIM].bitcast(i32)
                b3 = dg[:, :, DIM:2 * DIM].bitcast(i32)
                m3 = m[:, k * cslot:(k + 1) * cslot].unsqueeze(2).to_broadcast(
                    [128, cslot, DIM])
                nc.vector.tensor_tensor(t[:], a3, b3, op=x)
                nc.vector.tensor_tensor(t[:], t[:], m3, op=mybir.AluOpType.bitwise_and)
                nc.vector.tensor_tensor(dm[:].bitcast(i32), t[:], a3, op=x)
                oc = out_o[k * GC:(k + 1) * GC]
                nc.sync.dma_start(oc.rearrange("(s p) d -> p s d", p=128), dm[:])
            while ci < len(copies):
                nc.gpsimd.dma_start(*copies[ci]); ci += 1
    nc.compile()
    return nc


def _get_program():
    if "nc" not in _COMPILED:
        _COMPILED["nc"] = _build_program()
    return _COMPILED["nc"]


def kernel(weight, cuda_cached_weight, ids, idx_map, cached_idx_map, inverted_cached_idx):
    from concourse.bass_utils import run_bass_kernel_spmd

    weight = np.asarray(weight, dtype=np.float32)
    cache = np.asarray(cuda_cached_weight, dtype=np.float32)
    ids = np.asarray(ids, dtype=np.int32)
    idx_map = np.asarray(idx_map, dtype=np.int32)
    cached_idx_map = np.asarray(cached_idx_map, dtype=np.int32)
    inverted_cached_idx = np.asarray(inverted_cached_idx, dtype=np.int32)

    plan = _host_plan(ids, idx_map, cached_idx_map, inverted_cached_idx)

    if not _fast_path_ok(plan, cached_idx_map):
        out, cc, w = _host_fallback(weight, cache, plan)
        return out, cc, w, plan["cached_new"], plan["inv_new"]

    cri = plan["cpu_row_idxs"]
    nc = _get_program()

    ga_v = weight[F0:F1].reshape(WHALF, 2 * DIM)   # superrow view (free)
    in_maps = []
    for c in range(NCORE):
        src = cri[c * JPC : (c + 1) * JPC] - F0   # window-relative, [0, 65536)
        idxa_v = _wrap_idx((src >> 1).astype(np.int16))
        odd = (src & 1).astype(bool)
        flags = np.where(odd, np.int32(-1), np.int32(0)).reshape(S, 128).T
        msk_v = np.ascontiguousarray(flags)
        sb = E0 + (NCORE - 1 - c) * JPC           # this core's slot block start
        in_maps.append({
            "wa": weight[c * UWA : (c + 1) * UWA],
            "wb": weight[F0 + c * UWB : F0 + (c + 1) * UWB],
            "ca": cache[c * UWA : (c + 1) * UWA],
            "cs": cache[sb : sb + JPC],
            "ga": ga_v,
            "idxa": idxa_v,
            "msk": msk_v,
        })

    res = run_bass_kernel_spmd(nc, in_maps, list(range(NCORE)), trace=False)
    r = res.results

    weight_out = np.empty_like(weight)
    cache_out = np.empty_like(cache)
    out = np.empty((K, DIM), dtype=np.float32)
    for c in range(NCORE):
        sb = E0 + (NCORE - 1 - c) * JPC
        weight_out[c * UWA : (c + 1) * UWA] = r[c]["wa_o"]
        weight_out[F0 + c * UWB : F0 + (c + 1) * UWB] = r[c]["wb_o"]
        weight_out[sb : sb + JPC] = r[c]["we_o"]
        cache_out[c * UWA : (c + 1) * UWA] = r[c]["ca_o"]
        cache_out[sb : sb + JPC] = r[c]["out_o"][::-1]  # descending-j slots
        out[c * JPC : (c + 1) * JPC] = r[c]["out_o"]

    return out, cache_out, weight_out, plan["cached_new"], plan["inv_new"]
